# revision 9
# baseline (speedup 1.0000x reference)
"""Trainium2 Bass kernel for pair/all-pairs scoring.

Math (same decomposition as the reference):
    s1 = sent_feat @ W[:D],  s2 = sent_feat @ W[D:]
    all_score[i, j]  = s1[i] + s2[j] + b
    pair_score[e, k] = (s1[edge[e,0]] + b) + s2[edge[e,k+1]]

Sharding (8 cores, row-parallel):
    core c owns sent_feat rows [c*1024, (c+1)*1024) and edge rows likewise.
    Each core computes its local s1/s2 slab (fused DVE multiply+reduce), one
    8-core AllGather shares all slabs (16 KiB), the shared row is replicated
    across partitions with a K=1 ones-matmul on the TensorEngine, and each
    core then computes its [1024, 8192] slab of all_score (outer-sum split
    between the Vector and Scalar engines) and its 1024 rows of pair_score
    (GPSIMD ap_gather from the replicated score row).
"""

import sys

sys.path.insert(0, "/opt/trn_rl_repo")

import numpy as np

N = 8192  # sentences
D = 1024  # feature dim
E = 8192  # edge rows
K = 16  # 1 center + 15 neighbors
NCORES = 8
NLOC = N // NCORES  # 1024 rows per core
ELOC = E // NCORES  # 1024 edge rows per core
P = 128
NT = NLOC // P  # 8 feature tiles per core
JT = 1024  # all_score column tile (= one rank block)
EG = ELOC // P  # 8 gpsimd groups of 128 edges
NIDX = ELOC * K // EG  # 2048 gather indices per group
BC = 512  # ones-matmul broadcast chunk (PE moving-free limit)

_CACHE = {}


def _build(stage=4):
    """stage: 1=matvec only, 2=+exchange, 3=+main loop, 4=+pair (full)."""
    from concourse import bacc, mybir, tile

    f32 = mybir.dt.float32
    i16 = mybir.dt.int16

    nc = bacc.Bacc("TRN2", target_bir_lowering=False, debug=False, num_devices=NCORES)

    feat = nc.dram_tensor("feat", [NLOC, D], f32, kind="ExternalInput")
    wrep = nc.dram_tensor("wrep", [P, 2 * D + 1], f32, kind="ExternalInput")
    eidx = nc.dram_tensor("eidx", [16 * EG, NIDX // 16], i16, kind="ExternalInput")
    ident_in = nc.dram_tensor("ident", [P, P], f32, kind="ExternalInput")
    all_out = nc.dram_tensor("all_out", [NLOC, N], f32, kind="ExternalOutput")
    pair_out = nc.dram_tensor("pair_out", [ELOC, K - 1], f32, kind="ExternalOutput")

    MUL = mybir.AluOpType.mult

    with tile.TileContext(nc) as tc:
        with (
            tc.tile_pool(name="const", bufs=1) as constp,
            tc.tile_pool(name="featp", bufs=3) as featp,
            tc.tile_pool(name="prod", bufs=2) as prodp,
            tc.tile_pool(name="svec", bufs=1) as svecp,
            tc.tile_pool(name="big", bufs=1) as bigp,
            tc.tile_pool(name="outp", bufs=5) as outp,
            tc.tile_pool(name="gth", bufs=1) as gthp,
            tc.tile_pool(name="ps", bufs=4, space="PSUM") as psp,
            tc.tile_pool(name="pt", bufs=2, space="PSUM") as ptp,
            tc.tile_pool(name="dram", bufs=1, space="DRAM") as dramp,
        ):
            wsb = constp.tile([P, 2 * D + 1], f32)
            nc.sync.dma_start(wsb[:], wrep[:])
            idxsb = constp.tile([16 * EG, NIDX // 16], i16)
            nc.sync.dma_start(idxsb[:], eidx[:])
            ident = constp.tile([P, P], f32)
            nc.sync.dma_start(ident[:], ident_in[:])
            ones = constp.tile([1, P], f32)
            nc.vector.memset(ones[:], 1.0)

            # tiny dummy gather: hoists the gpsimd ucode library load (and its
            # engine barrier) to the start of the kernel, when everything is idle
            dsrc = gthp.tile([16, 8], f32, tag="dsrc")
            nc.vector.memset(dsrc[:], 0.0)
            didx = gthp.tile([16, 1], i16, tag="didx")
            nc.vector.memset(didx[:], 0)
            dout = gthp.tile([16, 16], f32, tag="dout")
            nc.gpsimd.ap_gather(
                dout[:], dsrc[:], didx[:], channels=16, num_elems=8, d=1, num_idxs=16
            )

            # ---- local matvec: fused multiply+accumulate along free dim ----
            s1p = svecp.tile([P, NT], f32)
            s2p = svecp.tile([P, NT], f32)
            for i in range(NT):
                ft = featp.tile([P, D], f32)
                nc.sync.dma_start(ft[:], feat[i * P : (i + 1) * P, :])
                p2 = prodp.tile([P, D], f32, tag="prod")
                nc.vector.scalar_tensor_tensor(
                    p2[:], ft[:], 1.0, wsb[:, D : 2 * D], MUL, MUL,
                    accum_out=s2p[:, i : i + 1],
                )
                p1 = prodp.tile([P, D], f32, tag="prod")
                nc.vector.scalar_tensor_tensor(
                    p1[:], ft[:], 1.0, wsb[:, 0:D], MUL, MUL,
                    accum_out=s1p[:, i : i + 1],
                )
            # fold the bias into s1
            nc.vector.tensor_add(
                s1p[:], s1p[:], wsb[:, 2 * D : 2 * D + 1].broadcast_to([P, NT])
            )

            # ---- exchange: transpose to free-major, single AllGather ----
            pt2 = ptp.tile([NT, P], f32, tag="pt")
            nc.tensor.transpose(pt2[:], s2p[:], ident[:])
            s2T = svecp.tile([NT, P], f32)
            nc.vector.tensor_copy(s2T[:], pt2[:])
            pt1 = ptp.tile([NT, P], f32, tag="pt")
            nc.tensor.transpose(pt1[:], s1p[:], ident[:])
            s1T = svecp.tile([NT, P], f32)
            nc.vector.tensor_copy(s1T[:], pt1[:])

            # per-rank contribution: [s2_own (1024) ; s1_own+b (1024)]
            cc_in = dramp.tile([2 * NT, P], f32, tag="cci")
            nc.sync.dma_start(cc_in[0:NT, :], s2T[:])
            nc.sync.dma_start(cc_in[NT : 2 * NT, :], s1T[:])
            cc_out = dramp.tile([2 * N], f32, tag="cco")
            nc.gpsimd.collective_compute(
                "AllGather",
                mybir.AluOpType.bypass,
                replica_groups=[list(range(NCORES))],
                ins=[cc_in.opt()],
                outs=[cc_out.opt()],
            )

            # ---- replicate the shared row across partitions (PE ones-matmul
            # + ACT/DVE copies), all blocks up front
            scg = bigp.tile([P, 2 * N], f32, tag="scg")
            nc.sync.dma_start(scg[0:1, :], cc_out.opt())
            for r in range(NCORES):
                base = r * 2 * NLOC
                for cpart in range(2 * NLOC // BC):
                    lo = base + cpart * BC
                    pb = psp.tile([P, BC], f32, tag="bc")
                    nc.tensor.matmul(
                        pb[:], ones[:], scg[0:1, lo : lo + BC], start=True, stop=True
                    )
                    if cpart % 2 == 0:
                        nc.vector.tensor_copy(scg[:, lo : lo + BC], pb[:])
                    else:
                        nc.scalar.copy(scg[:, lo : lo + BC], pb[:])

            # ---- pair_score gather: emitted before the main loop so the slow
            # gpsimd gather overlaps it; its DMAs ride the gpsimd SWDGE queue
            g = gthp.tile([16 * EG, NIDX], f32)
            nc.gpsimd.ap_gather(
                g[:],
                scg[0 : 16 * EG, :],
                idxsb[:],
                channels=16 * EG,
                num_elems=2 * N,
                d=1,
                num_idxs=NIDX,
            )
            # one partition per 16-partition group holds that group's values
            scratch = dramp.tile([EG, NIDX], f32, tag="scr")
            for gi in range(EG):
                nc.gpsimd.dma_start(
                    scratch[gi : gi + 1, :], g[16 * gi : 16 * gi + 1, :]
                )
            pg = gthp.tile([P, P], f32)
            nc.gpsimd.dma_start(
                pg[:], scratch.opt().rearrange("a (b f) -> (a b) f", f=P)
            )

            # ---- all_score main loop, split across DVE and ACT ----
            eng = 0
            for r in range(NCORES):
                base = r * 2 * NLOC
                for i in range(NT):
                    ot = outp.tile([P, JT], f32)
                    if eng % 2 == 0:
                        nc.vector.tensor_scalar_add(
                            ot[:], scg[:, base : base + NLOC], s1p[:, i : i + 1]
                        )
                    else:
                        nc.scalar.add(
                            ot[:], scg[:, base : base + NLOC], s1p[:, i : i + 1]
                        )
                    eng += 1
                    nc.sync.dma_start(
                        all_out[i * P : (i + 1) * P, r * JT : (r + 1) * JT], ot[:]
                    )

            # ---- pair tail: tiny add + store once pg is back ----
            pgv = pg[:].rearrange("p (m k) -> p m k", k=K)
            pairt = gthp.tile([P, (P // K) * (K - 1)], f32)
            pairtv = pairt[:].rearrange("p (m k) -> p m k", k=K - 1)
            nc.vector.tensor_add(
                pairtv,
                pgv[:, :, 1:K],
                pgv[:, :, 0:1].broadcast_to([P, P // K, K - 1]),
            )
            nc.sync.dma_start(
                pair_out.ap().rearrange("(a b) k -> a (b k)", b=P // K), pairt[:]
            )

    return _finish(nc)


def _finish(nc):
    nc.compile()
    return nc


def get_nc():
    if "nc" not in _CACHE:
        _CACHE["nc"] = _build()
    return _CACHE["nc"]


def host_inputs(sent_feat, W, b, edge):
    """Build the per-core input maps (pure marshalling: slab slicing, weight
    replication, and int16 gather-offset precomputation)."""
    sent_feat = np.ascontiguousarray(np.asarray(sent_feat, dtype=np.float32))
    W = np.asarray(W, dtype=np.float32).reshape(2 * D)
    bval = np.float32(np.asarray(b, dtype=np.float32).reshape(-1)[0])
    edge = np.asarray(edge).astype(np.int64).reshape(E, K)

    wrow = np.empty(2 * D + 1, dtype=np.float32)
    wrow[: 2 * D] = W
    wrow[2 * D] = bval
    wrep = np.ascontiguousarray(np.broadcast_to(wrow, (P, 2 * D + 1)))

    ident = np.eye(P, dtype=np.float32)

    # gather offsets into the AllGather layout:
    #   rank r block: [s2_own (NLOC) ; s1_own+b (NLOC)] at r*2*NLOC
    rank = edge // NLOC
    within = edge % NLOC
    off = rank * (2 * NLOC) + within  # s2 half
    off[:, 0] += NLOC  # center score lives in the s1 half
    in_maps = []
    for c in range(NCORES):
        loc = off[c * ELOC : (c + 1) * ELOC]  # [1024, 16]
        idx16 = (
            loc.reshape(EG, P, K).transpose(0, 2, 1).reshape(16 * EG, NIDX // 16)
        ).astype(np.int16)
        in_maps.append(
            {
                "feat": sent_feat[c * NLOC : (c + 1) * NLOC],
                "wrep": wrep,
                "eidx": np.ascontiguousarray(idx16),
                "ident": ident,
            }
        )
    return in_maps


def kernel(sent_feat, W, b, edge):
    from concourse.bass_utils import run_bass_kernel_spmd

    nc = get_nc()
    in_maps = host_inputs(sent_feat, W, b, edge)
    res = run_bass_kernel_spmd(nc, in_maps, list(range(NCORES)))
    all_score = np.concatenate(
        [np.asarray(res.results[c]["all_out"]) for c in range(NCORES)], axis=0
    )
    pair_score = np.concatenate(
        [np.asarray(res.results[c]["pair_out"]) for c in range(NCORES)], axis=0
    )
    return pair_score, all_score


# revision 13
# speedup vs baseline: 1.0728x; 1.0728x over previous
"""Trainium2 Bass kernel for pair/all-pairs scoring.

Math (same decomposition as the reference):
    s1 = sent_feat @ W[:D],  s2 = sent_feat @ W[D:]
    all_score[i, j]  = s1[i] + s2[j] + b
    pair_score[e, k] = (s1[edge[e,0]] + b) + s2[edge[e,k+1]]

Sharding (8 cores, row-parallel):
    core c owns sent_feat rows [c*1024, (c+1)*1024) and edge rows likewise.
    Each core computes its local s1/s2 slab (fused DVE multiply+reduce), one
    8-core AllGather shares all slabs (16 KiB), the shared row is replicated
    across partitions with a K=1 ones-matmul on the TensorEngine, and each
    core then computes its [1024, 8192] slab of all_score (outer-sum split
    between the Vector and Scalar engines) and its 1024 rows of pair_score
    (GPSIMD ap_gather from the replicated score row).
"""

import sys

sys.path.insert(0, "/opt/trn_rl_repo")

import numpy as np

N = 8192  # sentences
D = 1024  # feature dim
E = 8192  # edge rows
K = 16  # 1 center + 15 neighbors
NCORES = 8
NLOC = N // NCORES  # 1024 rows per core
ELOC = E // NCORES  # 1024 edge rows per core
P = 128
NT = NLOC // P  # 8 feature tiles per core
JT = 1024  # all_score column tile (= one rank block)
EG = ELOC // P  # 8 gpsimd groups of 128 edges
NIDX = ELOC * K // EG  # 2048 gather indices per group
BC = 512  # ones-matmul broadcast chunk (PE moving-free limit)

_CACHE = {}


def _build(stage=4):
    """stage: 1=matvec only, 2=+exchange, 3=+main loop, 4=+pair (full)."""
    from concourse import bacc, mybir, tile

    f32 = mybir.dt.float32
    i16 = mybir.dt.int16

    nc = bacc.Bacc("TRN2", target_bir_lowering=False, debug=False, num_devices=NCORES)

    feat = nc.dram_tensor("feat", [NLOC, D], f32, kind="ExternalInput")
    wrep = nc.dram_tensor("wrep", [P, 2 * D + 1], f32, kind="ExternalInput")
    eidx = nc.dram_tensor("eidx", [16 * EG, NIDX // 16], i16, kind="ExternalInput")
    ident_in = nc.dram_tensor("ident", [P, P], f32, kind="ExternalInput")
    all_out = nc.dram_tensor("all_out", [NLOC, N], f32, kind="ExternalOutput")
    pair_out = nc.dram_tensor("pair_out", [ELOC, K - 1], f32, kind="ExternalOutput")

    MUL = mybir.AluOpType.mult

    with tile.TileContext(nc) as tc:
        with (
            tc.tile_pool(name="const", bufs=1) as constp,
            tc.tile_pool(name="featp", bufs=3) as featp,
            tc.tile_pool(name="prod", bufs=2) as prodp,
            tc.tile_pool(name="svec", bufs=1) as svecp,
            tc.tile_pool(name="big", bufs=1) as bigp,
            tc.tile_pool(name="outp", bufs=8) as outp,
            tc.tile_pool(name="gth", bufs=1) as gthp,
            tc.tile_pool(name="ps", bufs=4, space="PSUM") as psp,
            tc.tile_pool(name="pt", bufs=2, space="PSUM") as ptp,
            tc.tile_pool(name="dram", bufs=1, space="DRAM") as dramp,
        ):
            wsb = constp.tile([P, 2 * D + 1], f32)
            nc.sync.dma_start(wsb[:], wrep[:])
            idxsb = constp.tile([16 * EG, NIDX // 16], i16)
            nc.sync.dma_start(idxsb[:], eidx[:])
            ident = constp.tile([P, P], f32)
            nc.sync.dma_start(ident[:], ident_in[:])
            ones = constp.tile([1, P], f32)
            nc.vector.memset(ones[:], 1.0)

            # the shared-row tile is allocated up front and cleared on the idle
            # Scalar engine so the gather never reads uninitialized partitions
            scg = bigp.tile([P, 2 * N], f32, tag="scg")
            nc.gpsimd.memset(scg[:, :], 0.0)

            # tiny dummy gather: hoists the gpsimd ucode library load (and its
            # engine barrier) to the start of the kernel, when everything is idle
            dsrc = gthp.tile([16, 8], f32, tag="dsrc")
            nc.vector.memset(dsrc[:], 0.0)
            didx = gthp.tile([16, 1], i16, tag="didx")
            nc.vector.memset(didx[:], 0)
            dout = gthp.tile([16, 16], f32, tag="dout")
            nc.gpsimd.ap_gather(
                dout[:], dsrc[:], didx[:], channels=16, num_elems=8, d=1, num_idxs=16
            )

            # ---- local matvec: fused multiply+accumulate along free dim ----
            s1p = svecp.tile([P, NT], f32)
            s2p = svecp.tile([P, NT], f32)
            for i in range(NT):
                ft = featp.tile([P, D], f32)
                nc.sync.dma_start(ft[:], feat[i * P : (i + 1) * P, :])
                p2 = prodp.tile([P, D], f32, tag="prod")
                nc.vector.scalar_tensor_tensor(
                    p2[:], ft[:], 1.0, wsb[:, D : 2 * D], MUL, MUL,
                    accum_out=s2p[:, i : i + 1],
                )
                p1 = prodp.tile([P, D], f32, tag="prod")
                nc.vector.scalar_tensor_tensor(
                    p1[:], ft[:], 1.0, wsb[:, 0:D], MUL, MUL,
                    accum_out=s1p[:, i : i + 1],
                )
            # fold the bias into s1
            nc.vector.tensor_add(
                s1p[:], s1p[:], wsb[:, 2 * D : 2 * D + 1].broadcast_to([P, NT])
            )

            # ---- exchange: transpose to free-major, single AllGather ----
            pt2 = ptp.tile([NT, P], f32, tag="pt")
            nc.tensor.transpose(pt2[:], s2p[:], ident[:])
            s2T = svecp.tile([NT, P], f32)
            nc.vector.tensor_copy(s2T[:], pt2[:])
            pt1 = ptp.tile([NT, P], f32, tag="pt")
            nc.tensor.transpose(pt1[:], s1p[:], ident[:])
            s1T = svecp.tile([NT, P], f32)
            nc.vector.tensor_copy(s1T[:], pt1[:])

            # per-rank contribution: [s2_own (1024) ; s1_own+b (1024)]
            cc_in = dramp.tile([2 * NT, P], f32, tag="cci")
            nc.sync.dma_start(cc_in[0:NT, :], s2T[:])
            nc.sync.dma_start(cc_in[NT : 2 * NT, :], s1T[:])
            cc_out = dramp.tile([2 * N], f32, tag="cco")
            nc.gpsimd.collective_compute(
                "AllGather",
                mybir.AluOpType.bypass,
                replica_groups=[list(range(NCORES))],
                ins=[cc_in.opt()],
                outs=[cc_out.opt()],
            )

            # ---- gather-source rows: each gpsimd core's first partition gets
            # the full shared row (covers the s1 half the replication skips)
            for gidx in range(EG):
                nc.sync.dma_start(scg[16 * gidx : 16 * gidx + 1, :], cc_out.opt())

            # ---- replicate only the s2 half of each rank block across all
            # partitions (PE ones-matmul + ACT/DVE copies)
            for r in range(NCORES):
                base = r * 2 * NLOC
                for cpart in range(NLOC // BC):
                    lo = base + cpart * BC
                    pb = psp.tile([P, BC], f32, tag="bc")
                    nc.tensor.matmul(
                        pb[:], ones[:], scg[0:1, lo : lo + BC], start=True, stop=True
                    )
                    if cpart % 2 == 0:
                        nc.vector.tensor_copy(scg[:, lo : lo + BC], pb[:])
                    else:
                        nc.scalar.copy(scg[:, lo : lo + BC], pb[:])

            # ---- pair_score gather: emitted before the main loop so the slow
            # gpsimd gather overlaps it; its DMAs ride the gpsimd SWDGE queue
            g = gthp.tile([16 * EG, NIDX], f32)
            nc.gpsimd.ap_gather(
                g[:],
                scg[0 : 16 * EG, :],
                idxsb[:],
                channels=16 * EG,
                num_elems=2 * N,
                d=1,
                num_idxs=NIDX,
            )
            scratch = dramp.tile([EG, NIDX], f32, tag="scr")
            for gi in range(EG):
                nc.gpsimd.dma_start(
                    scratch[gi : gi + 1, :], g[16 * gi : 16 * gi + 1, :]
                )
            pg = gthp.tile([P, P], f32)
            nc.gpsimd.dma_start(
                pg[:], scratch.opt().rearrange("a (b f) -> (a b) f", f=P)
            )

            # ---- all_score main loop, split across DVE and ACT ----
            eng = 0
            for r in range(NCORES):
                base = r * 2 * NLOC
                for i in range(NT):
                    ot = outp.tile([P, JT], f32)
                    if eng % 2 == 0:
                        nc.vector.tensor_scalar_add(
                            ot[:], scg[:, base : base + NLOC], s1p[:, i : i + 1]
                        )
                    else:
                        nc.scalar.add(
                            ot[:], scg[:, base : base + NLOC], s1p[:, i : i + 1]
                        )
                    eng += 1
                    nc.sync.dma_start(
                        all_out[i * P : (i + 1) * P, r * JT : (r + 1) * JT], ot[:]
                    )

            # ---- pair tail: tiny add + store once pg is back ----
            pgv = pg[:].rearrange("p (m k) -> p m k", k=K)
            pairt = gthp.tile([P, (P // K) * (K - 1)], f32)
            pairtv = pairt[:].rearrange("p (m k) -> p m k", k=K - 1)
            nc.vector.tensor_add(
                pairtv,
                pgv[:, :, 1:K],
                pgv[:, :, 0:1].broadcast_to([P, P // K, K - 1]),
            )
            nc.sync.dma_start(
                pair_out.ap().rearrange("(a b) k -> a (b k)", b=P // K), pairt[:]
            )

    return _finish(nc)


def _finish(nc):
    nc.compile()
    return nc


def get_nc():
    if "nc" not in _CACHE:
        _CACHE["nc"] = _build()
    return _CACHE["nc"]


def host_inputs(sent_feat, W, b, edge):
    """Build the per-core input maps (pure marshalling: slab slicing, weight
    replication, and int16 gather-offset precomputation)."""
    sent_feat = np.ascontiguousarray(np.asarray(sent_feat, dtype=np.float32))
    W = np.asarray(W, dtype=np.float32).reshape(2 * D)
    bval = np.float32(np.asarray(b, dtype=np.float32).reshape(-1)[0])
    edge = np.asarray(edge).astype(np.int64).reshape(E, K)

    wrow = np.empty(2 * D + 1, dtype=np.float32)
    wrow[: 2 * D] = W
    wrow[2 * D] = bval
    wrep = np.ascontiguousarray(np.broadcast_to(wrow, (P, 2 * D + 1)))

    ident = np.eye(P, dtype=np.float32)

    # gather offsets into the AllGather layout:
    #   rank r block: [s2_own (NLOC) ; s1_own+b (NLOC)] at r*2*NLOC
    rank = edge // NLOC
    within = edge % NLOC
    off = rank * (2 * NLOC) + within  # s2 half
    off[:, 0] += NLOC  # center score lives in the s1 half
    in_maps = []
    for c in range(NCORES):
        loc = off[c * ELOC : (c + 1) * ELOC]  # [1024, 16]
        idx16 = (
            loc.reshape(EG, P, K).transpose(0, 2, 1).reshape(16 * EG, NIDX // 16)
        ).astype(np.int16)
        in_maps.append(
            {
                "feat": sent_feat[c * NLOC : (c + 1) * NLOC],
                "wrep": wrep,
                "eidx": np.ascontiguousarray(idx16),
                "ident": ident,
            }
        )
    return in_maps


def kernel(sent_feat, W, b, edge):
    from concourse.bass_utils import run_bass_kernel_spmd

    nc = get_nc()
    in_maps = host_inputs(sent_feat, W, b, edge)
    res = run_bass_kernel_spmd(nc, in_maps, list(range(NCORES)))
    all_score = np.concatenate(
        [np.asarray(res.results[c]["all_out"]) for c in range(NCORES)], axis=0
    )
    pair_score = np.concatenate(
        [np.asarray(res.results[c]["pair_out"]) for c in range(NCORES)], axis=0
    )
    return pair_score, all_score


# revision 14
# speedup vs baseline: 1.1366x; 1.0595x over previous
"""Trainium2 Bass kernel for pair/all-pairs scoring.

Math (same decomposition as the reference):
    s1 = sent_feat @ W[:D],  s2 = sent_feat @ W[D:]
    all_score[i, j]  = s1[i] + s2[j] + b
    pair_score[e, k] = (s1[edge[e,0]] + b) + s2[edge[e,k+1]]

Sharding (8 cores, row-parallel):
    core c owns sent_feat rows [c*1024, (c+1)*1024) and edge rows likewise.
    Each core computes its local s1/s2 slab (fused DVE multiply+reduce), one
    8-core AllGather shares all slabs (16 KiB), the shared row is replicated
    across partitions with a K=1 ones-matmul on the TensorEngine, and each
    core then computes its [1024, 8192] slab of all_score (outer-sum split
    between the Vector and Scalar engines) and its 1024 rows of pair_score
    (GPSIMD ap_gather from the replicated score row).
"""

import sys

sys.path.insert(0, "/opt/trn_rl_repo")

import numpy as np

N = 8192  # sentences
D = 1024  # feature dim
E = 8192  # edge rows
K = 16  # 1 center + 15 neighbors
NCORES = 8
NLOC = N // NCORES  # 1024 rows per core
ELOC = E // NCORES  # 1024 edge rows per core
P = 128
NT = NLOC // P  # 8 feature tiles per core
JT = 1024  # all_score column tile (= one rank block)
EG = ELOC // P  # 8 gpsimd groups of 128 edges
NIDX = ELOC * K // EG  # 2048 gather indices per group
BC = 512  # ones-matmul broadcast chunk (PE moving-free limit)

_CACHE = {}


def _build(stage=4):
    """stage: 1=matvec only, 2=+exchange, 3=+main loop, 4=+pair (full)."""
    from concourse import bacc, mybir, tile

    f32 = mybir.dt.float32
    i16 = mybir.dt.int16

    nc = bacc.Bacc("TRN2", target_bir_lowering=False, debug=False, num_devices=NCORES)

    feat = nc.dram_tensor("feat", [NLOC, D], f32, kind="ExternalInput")
    wrep = nc.dram_tensor("wrep", [P, 2 * D + 1], f32, kind="ExternalInput")
    eidx = nc.dram_tensor("eidx", [16 * EG, NIDX // 16], i16, kind="ExternalInput")
    ident_in = nc.dram_tensor("ident", [P, P], f32, kind="ExternalInput")
    all_out = nc.dram_tensor("all_out", [NLOC, N], f32, kind="ExternalOutput")
    pair_out = nc.dram_tensor("pair_out", [ELOC, K - 1], f32, kind="ExternalOutput")

    MUL = mybir.AluOpType.mult

    with tile.TileContext(nc) as tc:
        with (
            tc.tile_pool(name="const", bufs=1) as constp,
            tc.tile_pool(name="featp", bufs=3) as featp,
            tc.tile_pool(name="prod", bufs=2) as prodp,
            tc.tile_pool(name="svec", bufs=1) as svecp,
            tc.tile_pool(name="big", bufs=1) as bigp,
            tc.tile_pool(name="outp", bufs=8) as outp,
            tc.tile_pool(name="gth", bufs=1) as gthp,
            tc.tile_pool(name="ps", bufs=3, space="PSUM") as psp,
            tc.tile_pool(name="pt", bufs=2, space="PSUM") as ptp,
            tc.tile_pool(name="dram", bufs=1, space="DRAM") as dramp,
        ):
            wsb = constp.tile([P, 2 * D + 1], f32)
            nc.sync.dma_start(wsb[:], wrep[:])
            idxsb = constp.tile([16 * EG, NIDX // 16], i16)
            nc.sync.dma_start(idxsb[:], eidx[:])
            ident = constp.tile([P, P], f32)
            nc.sync.dma_start(ident[:], ident_in[:])
            ones = constp.tile([1, P], f32)
            nc.vector.memset(ones[:], 1.0)

            # the shared-row tile is allocated up front and cleared on the idle
            # Scalar engine so the gather never reads uninitialized partitions
            scg = bigp.tile([P, 2 * N], f32, tag="scg")
            nc.gpsimd.memset(scg[:, :], 0.0)

            # tiny dummy gather: hoists the gpsimd ucode library load (and its
            # engine barrier) to the start of the kernel, when everything is idle
            dsrc = gthp.tile([16, 8], f32, tag="dsrc")
            nc.vector.memset(dsrc[:], 0.0)
            didx = gthp.tile([16, 1], i16, tag="didx")
            nc.vector.memset(didx[:], 0)
            dout = gthp.tile([16, 16], f32, tag="dout")
            nc.gpsimd.ap_gather(
                dout[:], dsrc[:], didx[:], channels=16, num_elems=8, d=1, num_idxs=16
            )

            # ---- local matvec: fused multiply+accumulate along free dim ----
            s1p = svecp.tile([P, NT], f32)
            s2p = svecp.tile([P, NT], f32)
            for i in range(NT):
                ft = featp.tile([P, D], f32)
                nc.sync.dma_start(ft[:], feat[i * P : (i + 1) * P, :])
                p2 = prodp.tile([P, D], f32, tag="prod")
                nc.vector.scalar_tensor_tensor(
                    p2[:], ft[:], 1.0, wsb[:, D : 2 * D], MUL, MUL,
                    accum_out=s2p[:, i : i + 1],
                )
                p1 = prodp.tile([P, D], f32, tag="prod")
                nc.vector.scalar_tensor_tensor(
                    p1[:], ft[:], 1.0, wsb[:, 0:D], MUL, MUL,
                    accum_out=s1p[:, i : i + 1],
                )
            # fold the bias into s1
            nc.vector.tensor_add(
                s1p[:], s1p[:], wsb[:, 2 * D : 2 * D + 1].broadcast_to([P, NT])
            )

            # ---- exchange: transpose to free-major, single AllGather ----
            pt2 = ptp.tile([NT, P], f32, tag="pt")
            nc.tensor.transpose(pt2[:], s2p[:], ident[:])
            s2T = svecp.tile([NT, P], f32)
            nc.vector.tensor_copy(s2T[:], pt2[:])
            pt1 = ptp.tile([NT, P], f32, tag="pt")
            nc.tensor.transpose(pt1[:], s1p[:], ident[:])
            s1T = svecp.tile([NT, P], f32)
            nc.vector.tensor_copy(s1T[:], pt1[:])

            # per-rank contribution: [s2_own (1024) ; s1_own+b (1024)]
            cc_in = dramp.tile([2 * NT, P], f32, tag="cci")
            nc.sync.dma_start(cc_in[0:NT, :], s2T[:])
            nc.sync.dma_start(cc_in[NT : 2 * NT, :], s1T[:])
            cc_out = dramp.tile([2 * N], f32, tag="cco")
            nc.gpsimd.collective_compute(
                "AllGather",
                mybir.AluOpType.bypass,
                replica_groups=[list(range(NCORES))],
                ins=[cc_in.opt()],
                outs=[cc_out.opt()],
            )

            # ---- gather-source rows: each gpsimd core's first partition gets
            # the full shared row; the gather depends only on these
            for gidx in range(EG):
                nc.sync.dma_start(scg[16 * gidx : 16 * gidx + 1, :], cc_out.opt())

            # ---- pair_score gather: emitted first so the slow gpsimd gather
            # overlaps everything; its DMAs ride the gpsimd SWDGE queue
            g = gthp.tile([16 * EG, NIDX], f32)
            nc.gpsimd.ap_gather(
                g[:],
                scg[0 : 16 * EG, :],
                idxsb[:],
                channels=16 * EG,
                num_elems=2 * N,
                d=1,
                num_idxs=NIDX,
            )
            scratch = dramp.tile([EG, NIDX], f32, tag="scr")
            for gi in range(EG):
                nc.gpsimd.dma_start(
                    scratch[gi : gi + 1, :], g[16 * gi : 16 * gi + 1, :]
                )
            pg = gthp.tile([P, P], f32)
            nc.gpsimd.dma_start(
                pg[:], scratch.opt().rearrange("a (b f) -> (a b) f", f=P)
            )

            # ---- all_score main loop: per rank, broadcast the s2 block into
            # PSUM with a K=1 ones-matmul, then DVE/ACT read PSUM directly
            eng = 0
            for r in range(NCORES):
                base = r * 2 * NLOC
                pbr = psp.tile([P, NLOC], f32, tag="bc")
                for cpart in range(NLOC // BC):
                    lo = cpart * BC
                    nc.tensor.matmul(
                        pbr[:, lo : lo + BC],
                        ones[:],
                        scg[0:1, base + lo : base + lo + BC],
                        start=True,
                        stop=True,
                    )
                for i in range(NT):
                    ot = outp.tile([P, JT], f32)
                    if eng % 2 == 0:
                        nc.vector.tensor_scalar_add(
                            ot[:], pbr[:, :], s1p[:, i : i + 1]
                        )
                    else:
                        nc.scalar.add(ot[:], pbr[:, :], s1p[:, i : i + 1])
                    eng += 1
                    nc.sync.dma_start(
                        all_out[i * P : (i + 1) * P, r * JT : (r + 1) * JT], ot[:]
                    )

            # ---- pair tail: tiny add + store once pg is back ----
            pgv = pg[:].rearrange("p (m k) -> p m k", k=K)
            pairt = gthp.tile([P, (P // K) * (K - 1)], f32)
            pairtv = pairt[:].rearrange("p (m k) -> p m k", k=K - 1)
            nc.vector.tensor_add(
                pairtv,
                pgv[:, :, 1:K],
                pgv[:, :, 0:1].broadcast_to([P, P // K, K - 1]),
            )
            nc.sync.dma_start(
                pair_out.ap().rearrange("(a b) k -> a (b k)", b=P // K), pairt[:]
            )

    return _finish(nc)


def _finish(nc):
    nc.compile()
    return nc


def get_nc():
    if "nc" not in _CACHE:
        _CACHE["nc"] = _build()
    return _CACHE["nc"]


def host_inputs(sent_feat, W, b, edge):
    """Build the per-core input maps (pure marshalling: slab slicing, weight
    replication, and int16 gather-offset precomputation)."""
    sent_feat = np.ascontiguousarray(np.asarray(sent_feat, dtype=np.float32))
    W = np.asarray(W, dtype=np.float32).reshape(2 * D)
    bval = np.float32(np.asarray(b, dtype=np.float32).reshape(-1)[0])
    edge = np.asarray(edge).astype(np.int64).reshape(E, K)

    wrow = np.empty(2 * D + 1, dtype=np.float32)
    wrow[: 2 * D] = W
    wrow[2 * D] = bval
    wrep = np.ascontiguousarray(np.broadcast_to(wrow, (P, 2 * D + 1)))

    ident = np.eye(P, dtype=np.float32)

    # gather offsets into the AllGather layout:
    #   rank r block: [s2_own (NLOC) ; s1_own+b (NLOC)] at r*2*NLOC
    rank = edge // NLOC
    within = edge % NLOC
    off = rank * (2 * NLOC) + within  # s2 half
    off[:, 0] += NLOC  # center score lives in the s1 half
    in_maps = []
    for c in range(NCORES):
        loc = off[c * ELOC : (c + 1) * ELOC]  # [1024, 16]
        idx16 = (
            loc.reshape(EG, P, K).transpose(0, 2, 1).reshape(16 * EG, NIDX // 16)
        ).astype(np.int16)
        in_maps.append(
            {
                "feat": sent_feat[c * NLOC : (c + 1) * NLOC],
                "wrep": wrep,
                "eidx": np.ascontiguousarray(idx16),
                "ident": ident,
            }
        )
    return in_maps


def kernel(sent_feat, W, b, edge):
    from concourse.bass_utils import run_bass_kernel_spmd

    nc = get_nc()
    in_maps = host_inputs(sent_feat, W, b, edge)
    res = run_bass_kernel_spmd(nc, in_maps, list(range(NCORES)))
    all_score = np.concatenate(
        [np.asarray(res.results[c]["all_out"]) for c in range(NCORES)], axis=0
    )
    pair_score = np.concatenate(
        [np.asarray(res.results[c]["pair_out"]) for c in range(NCORES)], axis=0
    )
    return pair_score, all_score
